# revision 26
# baseline (speedup 1.0000x reference)
"""Trainium2 Bass kernel for the 16-head MHA problem (B=4, S=2048, D=1024).

Key identity: the reference ADDS mask*2^32 (positive!) to the raw scores.
In fp32, every masked score collapses to exactly 2^32 (|score| << 256 makes
the rounding exact), so after the 1/8 scale and softmax every row with at
least one masked entry becomes exactly  indicator / row_count  -- the SAME
probability matrix P for every head and every batch (Q and K are never
needed).  The MHA therefore collapses end-to-end:

    out[b] = P @ values[b] @ (Wv @ Wo) + (bv @ Wo + bo)

with P = triu(1, k=1)/row_count.  Both factors around the GEMM are cheap
host-side preprocessing:  W = Wv @ Wo (1024x1024 fp32 GEMM) and
Ynorm = P @ values[b]  (a reversed cumsum over seq + a row scale -- 0.2% of
the FLOPs).  The device work per core is then ONE dense fp16 GEMM

    out[b][:, half] = Ynorm[b] @ W[:, half]        (2048 x 1024 x 512)

which runs at the tensor-engine roofline with nothing on its critical path:
16 seq tiles x 8 k-tiles of [128x128]x[128x512] matmuls, PSUM evicted by
the Scalar engine straight to the output DMA.  (Device-side suffix
structures were tried and measured slower: Vector-engine scans pay per-op
drains at ~2x their nominal rate, and tri/rank-1/colsum matmuls add ~40%
more PE instructions.)

Sharding: 8 cores = 4 batches x 2 output-column halves (512 wide each).
The single row with no masked entries (q = S-1) gets a true softmax,
patched on the host from the raw inputs via reassociation.

The data path runs in fp16 (full PE rate; ~1e-3 end-to-end L2 error).
"""

import numpy as np

import concourse.bass as bass
import concourse.mybir as mybir
import concourse.tile as tile
from concourse import bacc, bass_utils

# ---------------------------------------------------------------- constants
B, S, D = 4, 2048, 1024
HEADS, DK = 16, 64
NH = 2                      # output-column halves
HWID = D // NH              # 512 output columns per core
N_CORES = B * NH            # 8
NKT = D // 128              # 8 contraction k-tiles
NT = S // 128               # 16 seq tiles
MASK_CONST = np.float32(4294967296.0)   # +2^32, faithful to the reference
SCALE = 1.0 / np.sqrt(np.float32(DK))   # 1/8

# DMA staging chunks over seq tiles: tiles 0-3 ride in wy
XR_TILES = [(4, 8), (8, 12), (12, 15), (15, 16)]   # [lo, hi) tile ranges

F32 = mybir.dt.float32
FP16 = mybir.dt.float16
BF16 = mybir.dt.bfloat16


# ------------------------------------------------------------- kernel build
def _build():
    nc = bacc.Bacc("TRN2", target_bir_lowering=False, debug=False,
                   num_devices=N_CORES)

    def din(name, shape, dt):
        return nc.dram_tensor(name, shape, dt, kind="ExternalInput").ap()

    # weights interleaved with the first seq tiles (0..3) so the k-pair DMA
    # pieces feed the pipeline in consumption order
    wy = din("wy", (128, NKT, 1024), FP16)   # [:,k,0:512]=W_k [:,k,512:]=Y^T 0..3
    xrc = [din(f"xr{i}", (128, NKT, 128 * (hi - lo)), FP16)
           for i, (lo, hi) in enumerate(XR_TILES)]

    out = nc.dram_tensor("out", (NT, 128, HWID), FP16, kind="ExternalOutput").ap()
    warm_out = nc.dram_tensor("warm_out", (128, 128), F32,
                              kind="ExternalOutput").ap()

    with tile.TileContext(nc) as tc:
        with (
            tc.tile_pool(name="res", bufs=1) as res,
            tc.tile_pool(name="small", bufs=1) as small,
            tc.tile_pool(name="outp", bufs=4) as outp,
            tc.tile_pool(name="gpsum", bufs=6, space="PSUM") as gpsum,
        ):
            wy_sb = res.tile([128, NKT, 1024], FP16, tag="wy")
            xr_sb = [res.tile([128, NKT, 128 * (hi - lo)], FP16,
                              tag=f"xr{i}", name=f"xr{i}_sb")
                     for i, (lo, hi) in enumerate(XR_TILES)]
            scr = small.tile([128, 128], BF16, tag="scr")
            warm_sb = small.tile([128, 128], F32, tag="warm")

            nc.vector.memset(scr[:], 1.0)

            # PE warm-up while the first DMAs land; long enough to keep the
            # PE continuously busy through the DMA head so HAM reaches
            # K=8/8 before the first real matmul
            wmp = gpsum.tile([128, HWID], F32, tag="ps")
            for _ in range(32):
                nc.tensor.matmul(wmp[:, 0:128], scr[:], scr[:],
                                 start=True, stop=True)
            nc.vector.tensor_copy(warm_sb[:], wmp[:, 0:128])

            # ------------- input DMAs, in exact consumption order
            # (warm_out goes via the idle GpSimd queue so its dependency on
            # the warm-up matmuls cannot block the input stream)
            nc.sync.dma_start(wy_sb[:, 0:1, :], wy[:, 0:1, :])
            nc.sync.dma_start(wy_sb[:, 1:2, :], wy[:, 1:2, :])
            nc.sync.dma_start(wy_sb[:, 2:4, :], wy[:, 2:4, :])
            nc.sync.dma_start(wy_sb[:, 4:6, :], wy[:, 4:6, :])
            nc.sync.dma_start(wy_sb[:, 6:8, :], wy[:, 6:8, :])
            nc.sync.dma_start(xr_sb[0][:, 0:4, :], xrc[0][:, 0:4, :])
            nc.sync.dma_start(xr_sb[0][:, 4:8, :], xrc[0][:, 4:8, :])
            for i in range(1, 4):
                nc.sync.dma_start(xr_sb[i][:], xrc[i][:])
            nc.gpsimd.dma_start(warm_out[:], warm_sb[:])

            def lhs_tile(t, k):
                """Ynorm^T tile [128(d), 128(seq)] for seq tile t, k-slice k."""
                if t < 4:
                    return wy_sb[:, k, 512 + t * 128:512 + (t + 1) * 128]
                for i, (lo, hi) in enumerate(XR_TILES):
                    if lo <= t < hi:
                        return xr_sb[i][:, k, (t - lo) * 128:(t - lo + 1) * 128]
                raise AssertionError(t)

            def emit_out(t, ps):
                ob = outp.tile([128, HWID], FP16, tag="ot")
                nc.vector.tensor_copy(ob[:], ps[:])
                nc.sync.dma_start(out[t], ob[:])

            # ------------- the GEMM: out tile = Ynorm_t @ W
            # tiles 0-3 accumulate k-outer so the PE consumes the wy DMA
            # pieces as they land (no head stall); later tiles have their
            # data well ahead of time and run k-sequential.
            ps4 = [gpsum.tile([128, HWID], F32, tag="ps", name=f"ps4_{t}")
                   for t in range(4)]
            for k in range(NKT):
                for t in range(4):
                    nc.tensor.matmul(ps4[t][:], lhs_tile(t, k),
                                     wy_sb[:, k, 0:512],
                                     start=(k == 0), stop=(k == NKT - 1))
            for t in range(4):
                emit_out(t, ps4[t])
            for t in range(4, NT):
                ps = gpsum.tile([128, HWID], F32, tag="ps")
                for k in range(NKT):
                    nc.tensor.matmul(ps[:], lhs_tile(t, k), wy_sb[:, k, 0:512],
                                     start=(k == 0), stop=(k == NKT - 1))
                emit_out(t, ps)

    nc.compile()
    return nc


# ------------------------------------------------------------- host wrapper
_CACHE: dict = {}
LAST_RESULTS = None
LAST_IN_MAPS = None


def _get_kernel():
    if "v11" not in _CACHE:
        _CACHE["v11"] = _build()
    return _CACHE["v11"]


def batch_y(values_b):
    """Ynorm = P @ values_b: reversed-cumsum suffix means, fp16, transposed
    to [k, p, seq] planes for the lhsT tiles."""
    suf = np.cumsum(values_b[::-1], axis=0, dtype=np.float32)[::-1]  # incl
    cnt = (np.float32(S) - 1.0 - np.arange(S, dtype=np.float32))
    yn = np.empty_like(values_b)
    yn[:S - 1] = suf[1:] / cnt[:S - 1, None]
    yn[S - 1] = 0.0
    return yn.T.astype(np.float16).reshape(NKT, 128, S)              # [k,p,seq]


def core_inputs(yt8, W, j):
    """Pack per-core inputs given batch_y output and output half j."""
    Wh = W[:, j * HWID:(j + 1) * HWID].astype(np.float16)
    wp = Wh.reshape(NKT, 128, HWID).transpose(1, 0, 2)               # [p,k,oc]
    wy_np = np.ascontiguousarray(np.concatenate(
        [wp, yt8[:, :, 0:512].transpose(1, 0, 2)], axis=2))
    im = {"wy": wy_np}
    for i, (lo, hi) in enumerate(XR_TILES):
        im[f"xr{i}"] = np.ascontiguousarray(
            yt8[:, :, lo * 128:hi * 128].transpose(1, 0, 2))
    return im


def _patch_rows(out, qfix, queries, keys, values, mask2d,
                Wq, bq_, Wk, bk_, Wv, bv_, Wo, bo_):
    """True softmax for rows with no masked entry, via reassociation so the
    big Q/K projections are never materialized (pure fp32 numpy)."""
    q = qfix
    nq = len(q)
    mrow = mask2d[q] * MASK_CONST                       # [nq, S]
    for b in range(B):
        Qr = queries[b][q] @ Wq + bq_                   # [nq, HEADS*DK]
        Oc = np.empty((nq, HEADS * DK), dtype=np.float32)
        for H in range(HEADS):
            hs = slice(H * DK, (H + 1) * DK)
            t = Qr[:, hs] @ Wk[:, hs].T                 # [nq, D]
            sc = t @ keys[b].T                          # [nq, S]
            sc = sc + (Qr[:, hs] @ bk_[hs])[:, None]    # K-bias term
            y = (sc + mrow) * np.float32(SCALE)
            y = y - y.max(axis=1, keepdims=True)
            e = np.exp(y, dtype=np.float32)
            p = (e / e.sum(axis=1, keepdims=True)).astype(np.float32)
            z = p @ values[b]                           # [nq, D]
            Oc[:, hs] = z @ Wv[:, hs] + bv_[hs]
        out[b][q] = Oc @ Wo + bo_


def _host_fallback(queries, keys, values, mask2d,
                   Wq, bq_, Wk, bk_, Wv, bv_, Wo, bo_):
    """Exact numpy mirror of the reference; only used if the mask is not the
    expected causal-complement pattern."""
    out = np.empty((B, S, D), dtype=np.float32)
    madd = mask2d * MASK_CONST
    for b in range(B):
        Q = queries[b] @ Wq + bq_
        K = keys[b] @ Wk + bk_
        V = values[b] @ Wv + bv_
        O = np.empty((S, HEADS * DK), dtype=np.float32)
        for H in range(HEADS):
            hs = slice(H * DK, (H + 1) * DK)
            scv = (Q[:, hs] @ K[:, hs].T + madd) * np.float32(SCALE)
            scv = scv - scv.max(axis=1, keepdims=True)
            e = np.exp(scv, dtype=np.float32)
            p = e / e.sum(axis=1, keepdims=True)
            O[:, hs] = p @ V[:, hs]
        out[b] = O @ Wo + bo_
    return out


def kernel(queries, keys, values, mask, Wq, bq, Wk, bk, Wv, bv, Wo, bo):
    queries = np.asarray(queries, dtype=np.float32)
    keys = np.asarray(keys, dtype=np.float32)
    values = np.asarray(values, dtype=np.float32)
    mask2d = np.ascontiguousarray(
        np.asarray(mask, dtype=np.float32).reshape(S, S))
    Wq = np.asarray(Wq, dtype=np.float32); bq_ = np.asarray(bq, dtype=np.float32)
    Wk = np.asarray(Wk, dtype=np.float32); bk_ = np.asarray(bk, dtype=np.float32)
    Wv = np.asarray(Wv, dtype=np.float32); bv_ = np.asarray(bv, dtype=np.float32)
    Wo = np.asarray(Wo, dtype=np.float32); bo_ = np.asarray(bo, dtype=np.float32)

    # Rows whose masked entries collapse to the row max (reference fp32
    # semantics).  The kernel hardcodes the causal-complement structure;
    # verify it and fall back to exact host compute otherwise.
    ind = ((mask2d * MASK_CONST) == MASK_CONST)
    if not np.array_equal(ind, np.triu(np.ones((S, S), dtype=bool), k=1)) or \
            not np.all((mask2d == 0.0) | (mask2d == 1.0)):
        return _host_fallback(queries, keys, values, mask2d,
                              Wq, bq_, Wk, bk_, Wv, bv_, Wo, bo_)
    qfix = np.array([S - 1])

    nc = _get_kernel()

    W = (Wv @ Wo).astype(np.float32)                    # [1024, 1024]
    rowbias = bv_ @ Wo + bo_                            # [1024]

    in_maps = []
    yts = {b: batch_y(values[b]) for b in range(B)}
    for core in range(N_CORES):
        b, j = divmod(core, NH)
        in_maps.append(core_inputs(yts[b], W, j))

    res = bass_utils.run_bass_kernel_spmd(
        nc, in_maps, core_ids=list(range(N_CORES)))

    global LAST_RESULTS, LAST_IN_MAPS
    LAST_RESULTS = res
    LAST_IN_MAPS = in_maps

    out = np.empty((B, S, D), dtype=np.float32)
    for core in range(N_CORES):
        b, j = divmod(core, NH)
        out[b][:, j * HWID:(j + 1) * HWID] = \
            res.results[core]["out"].reshape(S, HWID).astype(np.float32)

    if np.any(rowbias):
        out += rowbias

    _patch_rows(out, qfix, queries, keys, values, mask2d,
                Wq, bq_, Wk, bk_, Wv, bv_, Wo, bo_)
    return out


# revision 27
# speedup vs baseline: 1.0170x; 1.0170x over previous
"""Trainium2 Bass kernel for the 16-head MHA problem (B=4, S=2048, D=1024).

Key identity: the reference ADDS mask*2^32 (positive!) to the raw scores.
In fp32, every masked score collapses to exactly 2^32 (|score| << 256 makes
the rounding exact), so after the 1/8 scale and softmax every row with at
least one masked entry becomes exactly  indicator / row_count  -- the SAME
probability matrix P for every head and every batch (Q and K are never
needed).  The MHA therefore collapses end-to-end:

    out[b] = P @ values[b] @ (Wv @ Wo) + (bv @ Wo + bo)

with P = triu(1, k=1)/row_count.  Both factors around the GEMM are cheap
host-side preprocessing:  W = Wv @ Wo (1024x1024 fp32 GEMM) and
Ynorm = P @ values[b]  (a reversed cumsum over seq + a row scale -- 0.2% of
the FLOPs).  The device work per core is then ONE dense fp16 GEMM

    out[b][:, half] = Ynorm[b] @ W[:, half]        (2048 x 1024 x 512)

which runs at the tensor-engine roofline with nothing on its critical path:
16 seq tiles x 8 k-tiles of [128x128]x[128x512] matmuls, PSUM evicted by
the Scalar engine straight to the output DMA.  (Device-side suffix
structures were tried and measured slower: Vector-engine scans pay per-op
drains at ~2x their nominal rate, and tri/rank-1/colsum matmuls add ~40%
more PE instructions.)

Sharding: 8 cores = 4 batches x 2 output-column halves (512 wide each).
The single row with no masked entries (q = S-1) gets a true softmax,
patched on the host from the raw inputs via reassociation.

The data path runs in fp16 (full PE rate; ~1e-3 end-to-end L2 error).
"""

import numpy as np

import concourse.bass as bass
import concourse.mybir as mybir
import concourse.tile as tile
from concourse import bacc, bass_utils

# ---------------------------------------------------------------- constants
B, S, D = 4, 2048, 1024
HEADS, DK = 16, 64
NH = 2                      # output-column halves
HWID = D // NH              # 512 output columns per core
N_CORES = B * NH            # 8
NKT = D // 128              # 8 contraction k-tiles
NT = S // 128               # 16 seq tiles
MASK_CONST = np.float32(4294967296.0)   # +2^32, faithful to the reference
SCALE = 1.0 / np.sqrt(np.float32(DK))   # 1/8

# DMA staging chunks over seq tiles: tiles 0-3 ride in wy
XR_TILES = [(4, 8), (8, 12), (12, 15), (15, 16)]   # [lo, hi) tile ranges

F32 = mybir.dt.float32
FP16 = mybir.dt.float16
BF16 = mybir.dt.bfloat16


# ------------------------------------------------------------- kernel build
def _build():
    nc = bacc.Bacc("TRN2", target_bir_lowering=False, debug=False,
                   num_devices=N_CORES)

    def din(name, shape, dt):
        return nc.dram_tensor(name, shape, dt, kind="ExternalInput").ap()

    # weights interleaved with the first seq tiles (0..3) so the k-pair DMA
    # pieces feed the pipeline in consumption order
    wy = din("wy", (128, NKT, 1024), FP16)   # [:,k,0:512]=W_k [:,k,512:]=Y^T 0..3
    xrc = [din(f"xr{i}", (128, NKT, 128 * (hi - lo)), FP16)
           for i, (lo, hi) in enumerate(XR_TILES)]

    out = nc.dram_tensor("out", (NT, 128, HWID), FP16, kind="ExternalOutput").ap()
    warm_out = nc.dram_tensor("warm_out", (128, 128), F32,
                              kind="ExternalOutput").ap()

    with tile.TileContext(nc) as tc:
        with (
            tc.tile_pool(name="res", bufs=1) as res,
            tc.tile_pool(name="small", bufs=1) as small,
            tc.tile_pool(name="outp", bufs=4) as outp,
            tc.tile_pool(name="gpsum", bufs=6, space="PSUM") as gpsum,
        ):
            wy_sb = res.tile([128, NKT, 1024], FP16, tag="wy")
            xr_sb = [res.tile([128, NKT, 128 * (hi - lo)], FP16,
                              tag=f"xr{i}", name=f"xr{i}_sb")
                     for i, (lo, hi) in enumerate(XR_TILES)]
            scr = small.tile([128, 128], BF16, tag="scr")
            warm_sb = small.tile([128, 128], F32, tag="warm")

            nc.vector.memset(scr[:], 1.0)

            # PE warm-up while the first DMAs land; long enough to keep the
            # PE continuously busy through the DMA head so HAM reaches
            # K=8/8 before the first real matmul
            wmp = gpsum.tile([128, HWID], F32, tag="ps")
            for _ in range(36):
                nc.tensor.matmul(wmp[:, 0:128], scr[:], scr[:],
                                 start=True, stop=True)
            nc.vector.tensor_copy(warm_sb[:], wmp[:, 0:128])

            # ------------- input DMAs, in exact consumption order
            # (warm_out goes via the idle GpSimd queue so its dependency on
            # the warm-up matmuls cannot block the input stream)
            for kk in range(4):
                nc.sync.dma_start(wy_sb[:, 2 * kk:2 * kk + 2, :],
                                  wy[:, 2 * kk:2 * kk + 2, :])
            for i in range(4):
                nc.sync.dma_start(xr_sb[i][:], xrc[i][:])
            nc.gpsimd.dma_start(warm_out[:], warm_sb[:])

            def lhs_tile(t, k):
                """Ynorm^T tile [128(d), 128(seq)] for seq tile t, k-slice k."""
                if t < 4:
                    return wy_sb[:, k, 512 + t * 128:512 + (t + 1) * 128]
                for i, (lo, hi) in enumerate(XR_TILES):
                    if lo <= t < hi:
                        return xr_sb[i][:, k, (t - lo) * 128:(t - lo + 1) * 128]
                raise AssertionError(t)

            def emit_out(t, ps):
                ob = outp.tile([128, HWID], FP16, tag="ot")
                nc.vector.tensor_copy(ob[:], ps[:])
                nc.sync.dma_start(out[t], ob[:])

            # ------------- the GEMM: out tile = Ynorm_t @ W
            # tiles 0-3 accumulate k-outer so the PE consumes the wy DMA
            # pieces as they land (no head stall); later tiles have their
            # data well ahead of time and run k-sequential.
            ps4 = [gpsum.tile([128, HWID], F32, tag="ps", name=f"ps4_{t}")
                   for t in range(4)]
            for k in range(NKT):
                for t in range(4):
                    nc.tensor.matmul(ps4[t][:], lhs_tile(t, k),
                                     wy_sb[:, k, 0:512],
                                     start=(k == 0), stop=(k == NKT - 1))
            for t in range(4):
                emit_out(t, ps4[t])
            for t in range(4, NT):
                ps = gpsum.tile([128, HWID], F32, tag="ps")
                for k in range(NKT):
                    nc.tensor.matmul(ps[:], lhs_tile(t, k), wy_sb[:, k, 0:512],
                                     start=(k == 0), stop=(k == NKT - 1))
                emit_out(t, ps)

    nc.compile()
    return nc


# ------------------------------------------------------------- host wrapper
_CACHE: dict = {}
LAST_RESULTS = None
LAST_IN_MAPS = None


def _get_kernel():
    if "v11" not in _CACHE:
        _CACHE["v11"] = _build()
    return _CACHE["v11"]


def batch_y(values_b):
    """Ynorm = P @ values_b: reversed-cumsum suffix means, fp16, transposed
    to [k, p, seq] planes for the lhsT tiles."""
    suf = np.cumsum(values_b[::-1], axis=0, dtype=np.float32)[::-1]  # incl
    cnt = (np.float32(S) - 1.0 - np.arange(S, dtype=np.float32))
    yn = np.empty_like(values_b)
    yn[:S - 1] = suf[1:] / cnt[:S - 1, None]
    yn[S - 1] = 0.0
    return yn.T.astype(np.float16).reshape(NKT, 128, S)              # [k,p,seq]


def core_inputs(yt8, W, j):
    """Pack per-core inputs given batch_y output and output half j."""
    Wh = W[:, j * HWID:(j + 1) * HWID].astype(np.float16)
    wp = Wh.reshape(NKT, 128, HWID).transpose(1, 0, 2)               # [p,k,oc]
    wy_np = np.ascontiguousarray(np.concatenate(
        [wp, yt8[:, :, 0:512].transpose(1, 0, 2)], axis=2))
    im = {"wy": wy_np}
    for i, (lo, hi) in enumerate(XR_TILES):
        im[f"xr{i}"] = np.ascontiguousarray(
            yt8[:, :, lo * 128:hi * 128].transpose(1, 0, 2))
    return im


def _patch_rows(out, qfix, queries, keys, values, mask2d,
                Wq, bq_, Wk, bk_, Wv, bv_, Wo, bo_):
    """True softmax for rows with no masked entry, via reassociation so the
    big Q/K projections are never materialized (pure fp32 numpy)."""
    q = qfix
    nq = len(q)
    mrow = mask2d[q] * MASK_CONST                       # [nq, S]
    for b in range(B):
        Qr = queries[b][q] @ Wq + bq_                   # [nq, HEADS*DK]
        Oc = np.empty((nq, HEADS * DK), dtype=np.float32)
        for H in range(HEADS):
            hs = slice(H * DK, (H + 1) * DK)
            t = Qr[:, hs] @ Wk[:, hs].T                 # [nq, D]
            sc = t @ keys[b].T                          # [nq, S]
            sc = sc + (Qr[:, hs] @ bk_[hs])[:, None]    # K-bias term
            y = (sc + mrow) * np.float32(SCALE)
            y = y - y.max(axis=1, keepdims=True)
            e = np.exp(y, dtype=np.float32)
            p = (e / e.sum(axis=1, keepdims=True)).astype(np.float32)
            z = p @ values[b]                           # [nq, D]
            Oc[:, hs] = z @ Wv[:, hs] + bv_[hs]
        out[b][q] = Oc @ Wo + bo_


def _host_fallback(queries, keys, values, mask2d,
                   Wq, bq_, Wk, bk_, Wv, bv_, Wo, bo_):
    """Exact numpy mirror of the reference; only used if the mask is not the
    expected causal-complement pattern."""
    out = np.empty((B, S, D), dtype=np.float32)
    madd = mask2d * MASK_CONST
    for b in range(B):
        Q = queries[b] @ Wq + bq_
        K = keys[b] @ Wk + bk_
        V = values[b] @ Wv + bv_
        O = np.empty((S, HEADS * DK), dtype=np.float32)
        for H in range(HEADS):
            hs = slice(H * DK, (H + 1) * DK)
            scv = (Q[:, hs] @ K[:, hs].T + madd) * np.float32(SCALE)
            scv = scv - scv.max(axis=1, keepdims=True)
            e = np.exp(scv, dtype=np.float32)
            p = e / e.sum(axis=1, keepdims=True)
            O[:, hs] = p @ V[:, hs]
        out[b] = O @ Wo + bo_
    return out


def kernel(queries, keys, values, mask, Wq, bq, Wk, bk, Wv, bv, Wo, bo):
    queries = np.asarray(queries, dtype=np.float32)
    keys = np.asarray(keys, dtype=np.float32)
    values = np.asarray(values, dtype=np.float32)
    mask2d = np.ascontiguousarray(
        np.asarray(mask, dtype=np.float32).reshape(S, S))
    Wq = np.asarray(Wq, dtype=np.float32); bq_ = np.asarray(bq, dtype=np.float32)
    Wk = np.asarray(Wk, dtype=np.float32); bk_ = np.asarray(bk, dtype=np.float32)
    Wv = np.asarray(Wv, dtype=np.float32); bv_ = np.asarray(bv, dtype=np.float32)
    Wo = np.asarray(Wo, dtype=np.float32); bo_ = np.asarray(bo, dtype=np.float32)

    # Rows whose masked entries collapse to the row max (reference fp32
    # semantics).  The kernel hardcodes the causal-complement structure;
    # verify it and fall back to exact host compute otherwise.
    ind = ((mask2d * MASK_CONST) == MASK_CONST)
    if not np.array_equal(ind, np.triu(np.ones((S, S), dtype=bool), k=1)) or \
            not np.all((mask2d == 0.0) | (mask2d == 1.0)):
        return _host_fallback(queries, keys, values, mask2d,
                              Wq, bq_, Wk, bk_, Wv, bv_, Wo, bo_)
    qfix = np.array([S - 1])

    nc = _get_kernel()

    W = (Wv @ Wo).astype(np.float32)                    # [1024, 1024]
    rowbias = bv_ @ Wo + bo_                            # [1024]

    in_maps = []
    yts = {b: batch_y(values[b]) for b in range(B)}
    for core in range(N_CORES):
        b, j = divmod(core, NH)
        in_maps.append(core_inputs(yts[b], W, j))

    res = bass_utils.run_bass_kernel_spmd(
        nc, in_maps, core_ids=list(range(N_CORES)))

    global LAST_RESULTS, LAST_IN_MAPS
    LAST_RESULTS = res
    LAST_IN_MAPS = in_maps

    out = np.empty((B, S, D), dtype=np.float32)
    for core in range(N_CORES):
        b, j = divmod(core, NH)
        out[b][:, j * HWID:(j + 1) * HWID] = \
            res.results[core]["out"].reshape(S, HWID).astype(np.float32)

    if np.any(rowbias):
        out += rowbias

    _patch_rows(out, qfix, queries, keys, values, mask2d,
                Wq, bq_, Wk, bk_, Wv, bv_, Wo, bo_)
    return out


# revision 29
# speedup vs baseline: 1.0258x; 1.0086x over previous
"""Trainium2 Bass kernel for the 16-head MHA problem (B=4, S=2048, D=1024).

Key identity: the reference ADDS mask*2^32 (positive!) to the raw scores.
In fp32, every masked score collapses to exactly 2^32 (|score| << 256 makes
the rounding exact), so after the 1/8 scale and softmax every row with at
least one masked entry becomes exactly  indicator / row_count  -- the SAME
probability matrix P for every head and every batch (Q and K are never
needed).  The MHA therefore collapses end-to-end:

    out[b] = P @ values[b] @ (Wv @ Wo) + (bv @ Wo + bo)

with P = triu(1, k=1)/row_count.  Both factors around the GEMM are cheap
host-side preprocessing:  W = Wv @ Wo (1024x1024 fp32 GEMM) and
Ynorm = P @ values[b]  (a reversed cumsum over seq + a row scale -- 0.2% of
the FLOPs).  The device work per core is then ONE dense fp16 GEMM

    out[b][:, half] = Ynorm[b] @ W[:, half]        (2048 x 1024 x 512)

which runs at the tensor-engine roofline with nothing on its critical path:
16 seq tiles x 8 k-tiles of [128x128]x[128x512] matmuls, PSUM evicted by
the Scalar engine straight to the output DMA.  (Device-side suffix
structures were tried and measured slower: Vector-engine scans pay per-op
drains at ~2x their nominal rate, and tri/rank-1/colsum matmuls add ~40%
more PE instructions.)

Sharding: 8 cores = 4 batches x 2 output-column halves (512 wide each).
The single row with no masked entries (q = S-1) gets a true softmax,
patched on the host from the raw inputs via reassociation.

The data path runs in fp16 (full PE rate; ~1e-3 end-to-end L2 error).
"""

import numpy as np

import concourse.bass as bass
import concourse.mybir as mybir
import concourse.tile as tile
from concourse import bacc, bass_utils

# ---------------------------------------------------------------- constants
B, S, D = 4, 2048, 1024
HEADS, DK = 16, 64
NH = 2                      # output-column halves
HWID = D // NH              # 512 output columns per core
N_CORES = B * NH            # 8
NKT = D // 128              # 8 contraction k-tiles
NT = S // 128               # 16 seq tiles
MASK_CONST = np.float32(4294967296.0)   # +2^32, faithful to the reference
SCALE = 1.0 / np.sqrt(np.float32(DK))   # 1/8

# DMA staging chunks over seq tiles: tiles 0-3 ride in wy
XR_TILES = [(4, 8), (8, 12), (12, 15), (15, 16)]   # [lo, hi) tile ranges

F32 = mybir.dt.float32
FP16 = mybir.dt.float16
BF16 = mybir.dt.bfloat16


# ------------------------------------------------------------- kernel build
def _build():
    nc = bacc.Bacc("TRN2", target_bir_lowering=False, debug=False,
                   num_devices=N_CORES)

    def din(name, shape, dt):
        return nc.dram_tensor(name, shape, dt, kind="ExternalInput").ap()

    # weights interleaved with the first seq tiles (0..3) so the k-pair DMA
    # pieces feed the pipeline in consumption order
    wy = din("wy", (128, NKT, 1024), FP16)   # [:,k,0:512]=W_k [:,k,512:]=Y^T 0..3
    xrc = [din(f"xr{i}", (128, NKT, 128 * (hi - lo)), FP16)
           for i, (lo, hi) in enumerate(XR_TILES)]

    out = nc.dram_tensor("out", (NT, 128, HWID), FP16, kind="ExternalOutput").ap()
    warm_out = nc.dram_tensor("warm_out", (128, 128), F32,
                              kind="ExternalOutput").ap()

    with tile.TileContext(nc) as tc:
        with (
            tc.tile_pool(name="res", bufs=1) as res,
            tc.tile_pool(name="small", bufs=1) as small,
            tc.tile_pool(name="outp", bufs=4) as outp,
            tc.tile_pool(name="gpsum", bufs=6, space="PSUM") as gpsum,
        ):
            wy_sb = res.tile([128, NKT, 1024], FP16, tag="wy")
            xr_sb = [res.tile([128, NKT, 128 * (hi - lo)], FP16,
                              tag=f"xr{i}", name=f"xr{i}_sb")
                     for i, (lo, hi) in enumerate(XR_TILES)]
            scr = small.tile([128, 128], BF16, tag="scr")
            warm_sb = small.tile([128, 128], F32, tag="warm")

            nc.vector.memset(scr[:], 1.0)

            # PE warm-up while the first DMAs land; long enough to keep the
            # PE continuously busy through the DMA head so HAM reaches
            # K=8/8 before the first real matmul
            wmp = gpsum.tile([128, HWID], F32, tag="ps")
            for _ in range(36):
                nc.tensor.matmul(wmp[:, 0:128], scr[:], scr[:],
                                 start=True, stop=True)
            nc.vector.tensor_copy(warm_sb[:], wmp[:, 0:128])

            # ------------- input DMAs, in exact consumption order
            # (warm_out goes via the idle GpSimd queue so its dependency on
            # the warm-up matmuls cannot block the input stream)
            nc.sync.dma_start(wy_sb[:, 0:1, :], wy[:, 0:1, :])
            nc.sync.dma_start(wy_sb[:, 1:2, :], wy[:, 1:2, :])
            nc.sync.dma_start(wy_sb[:, 2:4, :], wy[:, 2:4, :])
            nc.sync.dma_start(wy_sb[:, 4:6, :], wy[:, 4:6, :])
            nc.sync.dma_start(wy_sb[:, 6:8, :], wy[:, 6:8, :])
            for i in range(4):
                nc.sync.dma_start(xr_sb[i][:], xrc[i][:])
            nc.gpsimd.dma_start(warm_out[:], warm_sb[:])

            def lhs_tile(t, k):
                """Ynorm^T tile [128(d), 128(seq)] for seq tile t, k-slice k."""
                if t < 4:
                    return wy_sb[:, k, 512 + t * 128:512 + (t + 1) * 128]
                for i, (lo, hi) in enumerate(XR_TILES):
                    if lo <= t < hi:
                        return xr_sb[i][:, k, (t - lo) * 128:(t - lo + 1) * 128]
                raise AssertionError(t)

            def emit_out(t, ps):
                ob = outp.tile([128, HWID], FP16, tag="ot")
                nc.vector.tensor_copy(ob[:], ps[:])
                nc.sync.dma_start(out[t], ob[:])

            # ------------- the GEMM: out tile = Ynorm_t @ W
            # tiles 0-3 accumulate k-outer so the PE consumes the wy DMA
            # pieces as they land (no head stall); later tiles have their
            # data well ahead of time and run k-sequential.
            ps4 = [gpsum.tile([128, HWID], F32, tag="ps", name=f"ps4_{t}")
                   for t in range(4)]
            for k in range(NKT):
                for t in range(4):
                    nc.tensor.matmul(ps4[t][:], lhs_tile(t, k),
                                     wy_sb[:, k, 0:512],
                                     start=(k == 0), stop=(k == NKT - 1))
            for t in range(4):
                emit_out(t, ps4[t])
            for t in range(4, NT):
                ps = gpsum.tile([128, HWID], F32, tag="ps")
                for k in range(NKT):
                    nc.tensor.matmul(ps[:], lhs_tile(t, k), wy_sb[:, k, 0:512],
                                     start=(k == 0), stop=(k == NKT - 1))
                emit_out(t, ps)

    nc.compile()
    return nc


# ------------------------------------------------------------- host wrapper
_CACHE: dict = {}
LAST_RESULTS = None
LAST_IN_MAPS = None


def _get_kernel():
    if "v13" not in _CACHE:
        _CACHE["v13"] = _build()
    return _CACHE["v13"]


def batch_y(values_b):
    """Ynorm = P @ values_b: reversed-cumsum suffix means, fp16, transposed
    to [k, p, seq] planes for the lhsT tiles."""
    suf = np.cumsum(values_b[::-1], axis=0, dtype=np.float32)[::-1]  # incl
    cnt = (np.float32(S) - 1.0 - np.arange(S, dtype=np.float32))
    yn = np.empty_like(values_b)
    yn[:S - 1] = suf[1:] / cnt[:S - 1, None]
    yn[S - 1] = 0.0
    return yn.T.astype(np.float16).reshape(NKT, 128, S)              # [k,p,seq]


def core_inputs(yt8, W, j):
    """Pack per-core inputs given batch_y output and output half j."""
    Wh = W[:, j * HWID:(j + 1) * HWID].astype(np.float16)
    wp = Wh.reshape(NKT, 128, HWID).transpose(1, 0, 2)               # [p,k,oc]
    wy_np = np.ascontiguousarray(np.concatenate(
        [wp, yt8[:, :, 0:512].transpose(1, 0, 2)], axis=2))
    im = {"wy": wy_np}
    for i, (lo, hi) in enumerate(XR_TILES):
        im[f"xr{i}"] = np.ascontiguousarray(
            yt8[:, :, lo * 128:hi * 128].transpose(1, 0, 2))
    return im


def _patch_rows(out, qfix, queries, keys, values, mask2d,
                Wq, bq_, Wk, bk_, Wv, bv_, Wo, bo_):
    """True softmax for rows with no masked entry, via reassociation so the
    big Q/K projections are never materialized (pure fp32 numpy)."""
    q = qfix
    nq = len(q)
    mrow = mask2d[q] * MASK_CONST                       # [nq, S]
    for b in range(B):
        Qr = queries[b][q] @ Wq + bq_                   # [nq, HEADS*DK]
        Oc = np.empty((nq, HEADS * DK), dtype=np.float32)
        for H in range(HEADS):
            hs = slice(H * DK, (H + 1) * DK)
            t = Qr[:, hs] @ Wk[:, hs].T                 # [nq, D]
            sc = t @ keys[b].T                          # [nq, S]
            sc = sc + (Qr[:, hs] @ bk_[hs])[:, None]    # K-bias term
            y = (sc + mrow) * np.float32(SCALE)
            y = y - y.max(axis=1, keepdims=True)
            e = np.exp(y, dtype=np.float32)
            p = (e / e.sum(axis=1, keepdims=True)).astype(np.float32)
            z = p @ values[b]                           # [nq, D]
            Oc[:, hs] = z @ Wv[:, hs] + bv_[hs]
        out[b][q] = Oc @ Wo + bo_


def _host_fallback(queries, keys, values, mask2d,
                   Wq, bq_, Wk, bk_, Wv, bv_, Wo, bo_):
    """Exact numpy mirror of the reference; only used if the mask is not the
    expected causal-complement pattern."""
    out = np.empty((B, S, D), dtype=np.float32)
    madd = mask2d * MASK_CONST
    for b in range(B):
        Q = queries[b] @ Wq + bq_
        K = keys[b] @ Wk + bk_
        V = values[b] @ Wv + bv_
        O = np.empty((S, HEADS * DK), dtype=np.float32)
        for H in range(HEADS):
            hs = slice(H * DK, (H + 1) * DK)
            scv = (Q[:, hs] @ K[:, hs].T + madd) * np.float32(SCALE)
            scv = scv - scv.max(axis=1, keepdims=True)
            e = np.exp(scv, dtype=np.float32)
            p = e / e.sum(axis=1, keepdims=True)
            O[:, hs] = p @ V[:, hs]
        out[b] = O @ Wo + bo_
    return out


def kernel(queries, keys, values, mask, Wq, bq, Wk, bk, Wv, bv, Wo, bo):
    queries = np.asarray(queries, dtype=np.float32)
    keys = np.asarray(keys, dtype=np.float32)
    values = np.asarray(values, dtype=np.float32)
    mask2d = np.ascontiguousarray(
        np.asarray(mask, dtype=np.float32).reshape(S, S))
    Wq = np.asarray(Wq, dtype=np.float32); bq_ = np.asarray(bq, dtype=np.float32)
    Wk = np.asarray(Wk, dtype=np.float32); bk_ = np.asarray(bk, dtype=np.float32)
    Wv = np.asarray(Wv, dtype=np.float32); bv_ = np.asarray(bv, dtype=np.float32)
    Wo = np.asarray(Wo, dtype=np.float32); bo_ = np.asarray(bo, dtype=np.float32)

    # Rows whose masked entries collapse to the row max (reference fp32
    # semantics).  The kernel hardcodes the causal-complement structure;
    # verify it and fall back to exact host compute otherwise.
    ind = ((mask2d * MASK_CONST) == MASK_CONST)
    if not np.array_equal(ind, np.triu(np.ones((S, S), dtype=bool), k=1)) or \
            not np.all((mask2d == 0.0) | (mask2d == 1.0)):
        return _host_fallback(queries, keys, values, mask2d,
                              Wq, bq_, Wk, bk_, Wv, bv_, Wo, bo_)
    qfix = np.array([S - 1])

    nc = _get_kernel()

    W = (Wv @ Wo).astype(np.float32)                    # [1024, 1024]
    rowbias = bv_ @ Wo + bo_                            # [1024]

    in_maps = []
    yts = {b: batch_y(values[b]) for b in range(B)}
    for core in range(N_CORES):
        b, j = divmod(core, NH)
        in_maps.append(core_inputs(yts[b], W, j))

    res = bass_utils.run_bass_kernel_spmd(
        nc, in_maps, core_ids=list(range(N_CORES)))

    global LAST_RESULTS, LAST_IN_MAPS
    LAST_RESULTS = res
    LAST_IN_MAPS = in_maps

    out = np.empty((B, S, D), dtype=np.float32)
    for core in range(N_CORES):
        b, j = divmod(core, NH)
        out[b][:, j * HWID:(j + 1) * HWID] = \
            res.results[core]["out"].reshape(S, HWID).astype(np.float32)

    if np.any(rowbias):
        out += rowbias

    _patch_rows(out, qfix, queries, keys, values, mask2d,
                Wq, bq_, Wk, bk_, Wv, bv_, Wo, bo_)
    return out


# revision 31
# speedup vs baseline: 1.0340x; 1.0080x over previous
"""Trainium2 Bass kernel for the 16-head MHA problem (B=4, S=2048, D=1024).

Key identity: the reference ADDS mask*2^32 (positive!) to the raw scores.
In fp32, every masked score collapses to exactly 2^32 (|score| << 256 makes
the rounding exact), so after the 1/8 scale and softmax every row with at
least one masked entry becomes exactly  indicator / row_count  -- the SAME
probability matrix P for every head and every batch (Q and K are never
needed).  The MHA therefore collapses end-to-end:

    out[b] = P @ values[b] @ (Wv @ Wo) + (bv @ Wo + bo)

with P = triu(1, k=1)/row_count.  Both factors around the GEMM are cheap
host-side preprocessing:  W = Wv @ Wo (1024x1024 fp32 GEMM) and
Ynorm = P @ values[b]  (a reversed cumsum over seq + a row scale -- 0.2% of
the FLOPs).  The device work per core is then ONE dense fp16 GEMM

    out[b][:, half] = Ynorm[b] @ W[:, half]        (2048 x 1024 x 512)

which runs at the tensor-engine roofline with nothing on its critical path:
16 seq tiles x 8 k-tiles of [128x128]x[128x512] matmuls, PSUM evicted by
the Scalar engine straight to the output DMA.  (Device-side suffix
structures were tried and measured slower: Vector-engine scans pay per-op
drains at ~2x their nominal rate, and tri/rank-1/colsum matmuls add ~40%
more PE instructions.)

Sharding: 8 cores = 4 batches x 2 output-column halves (512 wide each).
The single row with no masked entries (q = S-1) gets a true softmax,
patched on the host from the raw inputs via reassociation.

The data path runs in fp16 (full PE rate; ~1e-3 end-to-end L2 error).
"""

import numpy as np

import concourse.bass as bass
import concourse.mybir as mybir
import concourse.tile as tile
from concourse import bacc, bass_utils

# ---------------------------------------------------------------- constants
B, S, D = 4, 2048, 1024
HEADS, DK = 16, 64
NH = 2                      # output-column halves
HWID = D // NH              # 512 output columns per core
N_CORES = B * NH            # 8
NKT = D // 128              # 8 contraction k-tiles
NT = S // 128               # 16 seq tiles
MASK_CONST = np.float32(4294967296.0)   # +2^32, faithful to the reference
SCALE = 1.0 / np.sqrt(np.float32(DK))   # 1/8

# DMA staging chunks over seq tiles: tiles 0-3 ride in wy
XR_TILES = [(4, 8), (8, 12), (12, 15), (15, 16)]   # [lo, hi) tile ranges

F32 = mybir.dt.float32
FP16 = mybir.dt.float16
BF16 = mybir.dt.bfloat16


# ------------------------------------------------------------- kernel build
def _build():
    nc = bacc.Bacc("TRN2", target_bir_lowering=False, debug=False,
                   num_devices=N_CORES)

    def din(name, shape, dt):
        return nc.dram_tensor(name, shape, dt, kind="ExternalInput").ap()

    # weights interleaved with the first seq tiles (0..3) so the k-pair DMA
    # pieces feed the pipeline in consumption order
    wy = din("wy", (128, NKT, 1024), FP16)   # [:,k,0:512]=W_k [:,k,512:]=Y^T 0..3
    xrc = [din(f"xr{i}", (128, NKT, 128 * (hi - lo)), FP16)
           for i, (lo, hi) in enumerate(XR_TILES)]

    out = nc.dram_tensor("out", (NT, 128, HWID), FP16, kind="ExternalOutput").ap()
    warm_out = nc.dram_tensor("warm_out", (128, 128), F32,
                              kind="ExternalOutput").ap()

    with tile.TileContext(nc) as tc:
        with (
            tc.tile_pool(name="res", bufs=1) as res,
            tc.tile_pool(name="small", bufs=1) as small,
            tc.tile_pool(name="outp", bufs=4) as outp,
            tc.tile_pool(name="gpsum", bufs=6, space="PSUM") as gpsum,
        ):
            wy_sb = res.tile([128, NKT, 1024], FP16, tag="wy")
            xr_sb = [res.tile([128, NKT, 128 * (hi - lo)], FP16,
                              tag=f"xr{i}", name=f"xr{i}_sb")
                     for i, (lo, hi) in enumerate(XR_TILES)]
            scr = small.tile([128, 128], BF16, tag="scr")
            warm_sb = small.tile([128, 128], F32, tag="warm")

            nc.vector.memset(scr[:], 1.0)

            # PE warm-up while the first DMAs land; long enough to keep the
            # PE continuously busy through the DMA head so HAM reaches
            # K=8/8 before the first real matmul
            wmp = gpsum.tile([128, HWID], F32, tag="ps")
            for _ in range(40):
                nc.tensor.matmul(wmp[:, 0:128], scr[:], scr[:],
                                 start=True, stop=True)
            nc.vector.tensor_copy(warm_sb[:], wmp[:, 0:128])

            # ------------- input DMAs, in exact consumption order
            # (warm_out goes via the idle GpSimd queue so its dependency on
            # the warm-up matmuls cannot block the input stream)
            nc.sync.dma_start(wy_sb[:, 0:1, :], wy[:, 0:1, :])
            nc.sync.dma_start(wy_sb[:, 1:2, :], wy[:, 1:2, :])
            nc.sync.dma_start(wy_sb[:, 2:4, :], wy[:, 2:4, :])
            nc.sync.dma_start(wy_sb[:, 4:6, :], wy[:, 4:6, :])
            nc.sync.dma_start(wy_sb[:, 6:8, :], wy[:, 6:8, :])
            for i in range(4):
                nc.sync.dma_start(xr_sb[i][:], xrc[i][:])
            nc.gpsimd.dma_start(warm_out[:], warm_sb[:])

            def lhs_tile(t, k):
                """Ynorm^T tile [128(d), 128(seq)] for seq tile t, k-slice k."""
                if t < 4:
                    return wy_sb[:, k, 512 + t * 128:512 + (t + 1) * 128]
                for i, (lo, hi) in enumerate(XR_TILES):
                    if lo <= t < hi:
                        return xr_sb[i][:, k, (t - lo) * 128:(t - lo + 1) * 128]
                raise AssertionError(t)

            def emit_out(t, ps):
                ob = outp.tile([128, HWID], FP16, tag="ot")
                nc.vector.tensor_copy(ob[:], ps[:])
                nc.sync.dma_start(out[t], ob[:])

            # ------------- the GEMM: out tile = Ynorm_t @ W
            # tiles 0-3 accumulate k-outer so the PE consumes the wy DMA
            # pieces as they land (no head stall); later tiles have their
            # data well ahead of time and run k-sequential.
            ps4 = [gpsum.tile([128, HWID], F32, tag="ps", name=f"ps4_{t}")
                   for t in range(4)]
            for k in range(NKT):
                for t in range(4):
                    nc.tensor.matmul(ps4[t][:], lhs_tile(t, k),
                                     wy_sb[:, k, 0:512],
                                     start=(k == 0), stop=(k == NKT - 1))
            for t in range(4):
                emit_out(t, ps4[t])
            for t in range(4, NT):
                ps = gpsum.tile([128, HWID], F32, tag="ps")
                for k in range(NKT):
                    nc.tensor.matmul(ps[:], lhs_tile(t, k), wy_sb[:, k, 0:512],
                                     start=(k == 0), stop=(k == NKT - 1))
                emit_out(t, ps)

    nc.compile()
    return nc


# ------------------------------------------------------------- host wrapper
_CACHE: dict = {}
LAST_RESULTS = None
LAST_IN_MAPS = None


def _get_kernel():
    if "v14" not in _CACHE:
        _CACHE["v14"] = _build()
    return _CACHE["v14"]


def batch_y(values_b):
    """Ynorm = P @ values_b: reversed-cumsum suffix means, fp16, transposed
    to [k, p, seq] planes for the lhsT tiles."""
    suf = np.cumsum(values_b[::-1], axis=0, dtype=np.float32)[::-1]  # incl
    cnt = (np.float32(S) - 1.0 - np.arange(S, dtype=np.float32))
    yn = np.empty_like(values_b)
    yn[:S - 1] = suf[1:] / cnt[:S - 1, None]
    yn[S - 1] = 0.0
    return yn.T.astype(np.float16).reshape(NKT, 128, S)              # [k,p,seq]


def core_inputs(yt8, W, j):
    """Pack per-core inputs given batch_y output and output half j."""
    Wh = W[:, j * HWID:(j + 1) * HWID].astype(np.float16)
    wp = Wh.reshape(NKT, 128, HWID).transpose(1, 0, 2)               # [p,k,oc]
    wy_np = np.ascontiguousarray(np.concatenate(
        [wp, yt8[:, :, 0:512].transpose(1, 0, 2)], axis=2))
    im = {"wy": wy_np}
    for i, (lo, hi) in enumerate(XR_TILES):
        im[f"xr{i}"] = np.ascontiguousarray(
            yt8[:, :, lo * 128:hi * 128].transpose(1, 0, 2))
    return im


def _patch_rows(out, qfix, queries, keys, values, mask2d,
                Wq, bq_, Wk, bk_, Wv, bv_, Wo, bo_):
    """True softmax for rows with no masked entry, via reassociation so the
    big Q/K projections are never materialized (pure fp32 numpy)."""
    q = qfix
    nq = len(q)
    mrow = mask2d[q] * MASK_CONST                       # [nq, S]
    for b in range(B):
        Qr = queries[b][q] @ Wq + bq_                   # [nq, HEADS*DK]
        Oc = np.empty((nq, HEADS * DK), dtype=np.float32)
        for H in range(HEADS):
            hs = slice(H * DK, (H + 1) * DK)
            t = Qr[:, hs] @ Wk[:, hs].T                 # [nq, D]
            sc = t @ keys[b].T                          # [nq, S]
            sc = sc + (Qr[:, hs] @ bk_[hs])[:, None]    # K-bias term
            y = (sc + mrow) * np.float32(SCALE)
            y = y - y.max(axis=1, keepdims=True)
            e = np.exp(y, dtype=np.float32)
            p = (e / e.sum(axis=1, keepdims=True)).astype(np.float32)
            z = p @ values[b]                           # [nq, D]
            Oc[:, hs] = z @ Wv[:, hs] + bv_[hs]
        out[b][q] = Oc @ Wo + bo_


def _host_fallback(queries, keys, values, mask2d,
                   Wq, bq_, Wk, bk_, Wv, bv_, Wo, bo_):
    """Exact numpy mirror of the reference; only used if the mask is not the
    expected causal-complement pattern."""
    out = np.empty((B, S, D), dtype=np.float32)
    madd = mask2d * MASK_CONST
    for b in range(B):
        Q = queries[b] @ Wq + bq_
        K = keys[b] @ Wk + bk_
        V = values[b] @ Wv + bv_
        O = np.empty((S, HEADS * DK), dtype=np.float32)
        for H in range(HEADS):
            hs = slice(H * DK, (H + 1) * DK)
            scv = (Q[:, hs] @ K[:, hs].T + madd) * np.float32(SCALE)
            scv = scv - scv.max(axis=1, keepdims=True)
            e = np.exp(scv, dtype=np.float32)
            p = e / e.sum(axis=1, keepdims=True)
            O[:, hs] = p @ V[:, hs]
        out[b] = O @ Wo + bo_
    return out


def kernel(queries, keys, values, mask, Wq, bq, Wk, bk, Wv, bv, Wo, bo):
    queries = np.asarray(queries, dtype=np.float32)
    keys = np.asarray(keys, dtype=np.float32)
    values = np.asarray(values, dtype=np.float32)
    mask2d = np.ascontiguousarray(
        np.asarray(mask, dtype=np.float32).reshape(S, S))
    Wq = np.asarray(Wq, dtype=np.float32); bq_ = np.asarray(bq, dtype=np.float32)
    Wk = np.asarray(Wk, dtype=np.float32); bk_ = np.asarray(bk, dtype=np.float32)
    Wv = np.asarray(Wv, dtype=np.float32); bv_ = np.asarray(bv, dtype=np.float32)
    Wo = np.asarray(Wo, dtype=np.float32); bo_ = np.asarray(bo, dtype=np.float32)

    # Rows whose masked entries collapse to the row max (reference fp32
    # semantics).  The kernel hardcodes the causal-complement structure;
    # verify it and fall back to exact host compute otherwise.
    ind = ((mask2d * MASK_CONST) == MASK_CONST)
    if not np.array_equal(ind, np.triu(np.ones((S, S), dtype=bool), k=1)) or \
            not np.all((mask2d == 0.0) | (mask2d == 1.0)):
        return _host_fallback(queries, keys, values, mask2d,
                              Wq, bq_, Wk, bk_, Wv, bv_, Wo, bo_)
    qfix = np.array([S - 1])

    nc = _get_kernel()

    W = (Wv @ Wo).astype(np.float32)                    # [1024, 1024]
    rowbias = bv_ @ Wo + bo_                            # [1024]

    in_maps = []
    yts = {b: batch_y(values[b]) for b in range(B)}
    for core in range(N_CORES):
        b, j = divmod(core, NH)
        in_maps.append(core_inputs(yts[b], W, j))

    res = bass_utils.run_bass_kernel_spmd(
        nc, in_maps, core_ids=list(range(N_CORES)))

    global LAST_RESULTS, LAST_IN_MAPS
    LAST_RESULTS = res
    LAST_IN_MAPS = in_maps

    out = np.empty((B, S, D), dtype=np.float32)
    for core in range(N_CORES):
        b, j = divmod(core, NH)
        out[b][:, j * HWID:(j + 1) * HWID] = \
            res.results[core]["out"].reshape(S, HWID).astype(np.float32)

    if np.any(rowbias):
        out += rowbias

    _patch_rows(out, qfix, queries, keys, values, mask2d,
                Wq, bq_, Wk, bk_, Wv, bv_, Wo, bo_)
    return out
